# revision 18
# baseline (speedup 1.0000x reference)
"""NT-Xent (SimCLR) contrastive loss on 8 Trainium2 NeuronCores — v2.

Reference (B=4096, D=256, T=0.5):
    z   = concat(l2norm(x_i), l2norm(x_j))        # [8192, 256]
    sim = z @ z.T
    loss = mean(-pos/T + log(sum_{j!=r} exp(sim_rj/T)))

v2 redesign vs v1 (124.3us):
  * Host supplies x TRANSPOSED (xt [256, 8192] bf16, rolled by -c*1024
    per core) — pure layout prep.  This kills the on-device
    normalize -> DRAM store -> DMA-transpose round trip that generated
    ~25K tiny DMA descriptors and delayed the first matmul to 35us.
  * Column norms computed ON the transposed layout with a ones-matmul
    partition reduction: sq8 = (x/16)*x in fp8 (gpsimd), then
    ones[128,2,128]^T (x) sq8 -> PSUM [128, cw] holding norm^2/16
    replicated across all partitions (exactly the free-dim multiplier
    shape the scale pass needs — no broadcast/gather dance).
  * Quake rsqrt on the replicated PSUM bits, output DIRECTLY as bf16
    bits ((MAGIC - bits>>1) >> 16 done as bits>>17 vs MAGIC>>16; the
    dropped borrow is +-1 bf16 ulp noise).  2 DVE ops per chunk.
  * zT8 = fp8e4(x * 4.0692 * u) — fp8 with a 16x scale (PSUM = 256*sim).
  * Matmul in fp8 DoubleRow perf mode: K=256 contracted in ONE pass
    ([128, 2, N] APs), ~1.7x over bf16.  64 (m,gc) tiles of
    [128 rows x 1024 cols], 2 MM instrs each (PSUM bank = 512 fp32).
  * exp+rowsum drain split across ACT (Exp, accum_out), DVE
    (int16-Schraudolph bf16 bit-trick), and gpsimd (2nd half of the
    Schraudolph on SBUF data — gpsimd has no PSUM port).
  * pos = diag(sim, +-B) extracted from the gc4 PSUM tiles with one
    tensor_tensor_reduce against an identity mask (diagonal structure
    survives the roll).  No row-major zhat needed at all.

Loss assembly: out[p] per core partition, host sums all cores / 8192.
"""

import os

import numpy as np

MM_MODE = os.environ.get("V2_MM", "dr")      # 'dr' | 'fp8'
POOL_TT = os.environ.get("V2_POOL", "1") == "1"
DO_POS = os.environ.get("V2_POS", "1") == "1"
DO_SCH = os.environ.get("V2_SCH", "1") == "1"

P = 128
D = 256
B = 4096
R = 2 * B                  # 8192 rows of z
NCORES = 8
BLK = R // NCORES          # 1024 rows per core
KO = 2                     # K chunks of 128 (D = 256)
CW = 2048                  # column chunk width (PSUM tile free dim)
NCH = R // CW              # 4 chunks / column groups
PB = 512                   # PSUM bank width in fp32 (matmul write unit)
MT = BLK // P              # 8 row tiles per core
T_INV = 2.0
E2 = float(np.exp(T_INV))
FP8_S = 16.0               # scale folded into fp8 z
PS_S = FP8_S * FP8_S       # PSUM = 256 * sim
EXP_SCALE = T_INV / PS_S   # ACT Exp scale
RSQRT_MAGIC = 0x5F3759DF
# Quake magic with log2(16*1.0173)=4.0248 exponent-bits folded in:
# ubits = MAGICF - bits(norm^2)/2, computed as ONE tensor_scalar
# (bits * -0.5 + MAGICF) — the f32-internal halving of the int rounds
# within quake noise.  bitcast(ubits) ~ 16.28/norm.
MAGICF = float(RSQRT_MAGIC + round(4.0248 * (1 << 23)))

# f32 Schraudolph: f32 bits of exp(y) ~= y*(2^23/ln2) + SCH_B with
# y = PSUM * 2/256.  -486411 is the classic centering; +2.5 bf16-bits
# (163840 f32-bits) zeroes the MEAN error over sim ~ N(0, 1/16) — what
# the denominator sums see (calibrated offline on synthetic N(0,1)).
SCH_A = float((1 << 23) / np.log(2.0) * EXP_SCALE)
SCH_B = float(127 * (1 << 23) - 486411.0 + 2.5 * 65536.0)

# drain engine per flat tile idx = gc*8+m (32 tiles of [128, 2048]):
#   'A' = ACT Exp, 'D' = DVE Schraudolph (2 passes)
# gc0 all ACT (DVE busy with chunk prep early); later ~30% DVE.
def _default_drains():
    # The ACT Exp chain is the wall (32 x 2.06us back-to-back).  DVE
    # drains cost ~4.5us/tile but DVE is idle once chunk preps finish
    # (~45us), so alternate ACT/DVE for gc2+gc3 ONLY — those drains sit
    # after all prep ops in the in-order DVE queue, so they cannot delay
    # the zt8 scale chain the way early DVE drains did.
    # Sparse D placement in gc2/gc3 (one per ~3 ACT turns measured best;
    # denser alternation starves the 2-slot PSUM rotation on pass1
    # completions), none on the last tiles where drain latency lands
    # directly on the exec time.  pass2 is deferred past the MM loop.
    return ["D" if i in (17, 20, 23, 26, 29) else "A" for i in range(32)]

DRAINS = _default_drains()
ND = max(1, DRAINS.count("D"))

_cached = None


def _build():
    import concourse.bacc as bacc
    import concourse.mybir as mybir
    from concourse import tile
    from concourse.masks import make_identity

    f32 = mybir.dt.float32
    bf16 = mybir.dt.bfloat16
    f8 = mybir.dt.float8e4
    i32 = mybir.dt.int32
    i16 = mybir.dt.int16
    AF = mybir.ActivationFunctionType
    ALU = mybir.AluOpType
    DR = mybir.MatmulPerfMode.DoubleRow

    # Pin Exp/Ln to the one table containing both -> single table load.
    from concourse import hw_specs as _hw

    _orig_gat = _hw.get_activation_tables

    def _gat_patched(arch):
        tabs = _orig_gat(arch)
        for name, fns in tabs.items():
            if name != "natural_log_exp_and_others":
                fns.discard(AF.Exp)
                fns.discard(AF.Ln)
        return tabs

    bacc.get_activation_tables = _gat_patched

    nc = bacc.Bacc(None, target_bir_lowering=False, debug=False)
    xt_d = nc.dram_tensor("xt", [D, R], bf16, kind="ExternalInput")
    out_d = nc.dram_tensor("out", [P, 1], f32, kind="ExternalOutput")

    def _emit(tc):
        with (
            tc.tile_pool(name="big", bufs=1) as big,
            tc.tile_pool(name="small", bufs=1) as small,
            tc.tile_pool(name="sqp", bufs=2) as sqp,
            tc.tile_pool(name="shp", bufs=2) as shp,
            tc.tile_pool(name="escp", bufs=3) as escp,
            tc.tile_pool(name="itp", bufs=2) as itp,
            tc.tile_pool(name="scratch", bufs=2) as scratch,
            tc.tile_pool(name="psum", bufs=1, space="PSUM") as psum,
        ):
            xts = big.tile([P, KO, R], bf16, name="xts")
            zt8 = big.tile([P, KO, R], f8, name="zt8")
            acc = small.tile([P, MT * NCH], f32, name="acc")
            posd = small.tile([P, MT], f32, name="posd")
            ident = small.tile([P, P], bf16, name="ident")
            ones8 = small.tile([P, KO, P], bf16, name="ones8")

            nc.gpsimd.memset(ones8[:], 1.0)
            nc.vector.memset(posd[:], 0.0)
            if DO_POS:
                make_identity(nc, ident[:])

            # all chunk loads up front on the sync DMA queue, one DMA per
            # (chunk, k) so the per-k squares can start on partial data
            for s in range(NCH):
                cs = slice(s * CW, (s + 1) * CW)
                for k in range(KO):
                    nc.sync.dma_start(
                        xts[:, k, cs], xt_d[k * P:(k + 1) * P, cs])

            chunk_psu = {}
            deferred = []

            def emit_chunk(s):
                cs = slice(s * CW, (s + 1) * CW)
                # sq = x^2 in bf16 (per-k 2D ops; late chunks on Pool —
                # slow engine, but off the early critical path)
                sq = sqp.tile([P, KO, CW], bf16, name="sq", tag="sq", bufs=2)
                sq_eng = nc.gpsimd if (POOL_TT and s >= 2) else nc.vector
                nh = 2 if s == 0 else 1
                for h in range(nh):
                    hw = CW // nh
                    for k in range(KO):
                        sq_eng.tensor_tensor(
                            out=sq[:, k, h * hw:(h + 1) * hw],
                            in0=xts[:, k, s * CW + h * hw:s * CW + (h + 1) * hw],
                            in1=xts[:, k, s * CW + h * hw:s * CW + (h + 1) * hw],
                            op=ALU.mult)
                # ones^T (x) sq -> norm^2 replicated on all partitions
                # (psu joins the main PSUM tile rotation)
                psu = psum.tile([P, CW], f32, name="ps", tag="ps", bufs=2)
                for b in range(CW // PB):
                    bs = slice(b * PB, (b + 1) * PB)
                    for k in range(KO):
                        nc.tensor.matmul(
                            psu[:, bs], ones8[:, k, :], sq[:, k, bs],
                            start=(k == 0), stop=(k == KO - 1))
                chunk_psu[s] = psu

            def emit_uq(s):
                psu = chunk_psu[s]
                # Quake rsqrt in ONE fast-class DVE op (f32/i32 in+out);
                # chunk 0 split in halves to cut the fill-chain latency
                ub = shp.tile([P, CW], i32, name="ub", tag="ub", bufs=2)
                nq = 2 if s == 0 else 1
                for h in range(nq):
                    qw = CW // nq
                    qs = slice(h * qw, (h + 1) * qw)
                    nc.vector.tensor_scalar(
                        out=ub[:, qs], in0=psu[:, qs].bitcast(i32),
                        scalar1=-0.5, scalar2=MAGICF,
                        op0=ALU.mult, op1=ALU.add)
                # zt8 = fp8(x * 16.28/norm) — per-k 2D TT multiply; chunk 0
                # split into halves so gc0's first matmuls start sooner
                # all scales on DVE: chunk3's scale on Pool ran ~8us and
                # landed right before gc3 needed it, stalling ACT ~4us;
                # DVE is idle by then and finishes it sooner
                sc_eng = nc.vector
                nh = 2 if s == 0 else 1
                for h in range(nh):
                    hw = CW // nh
                    hs = slice(s * CW + h * hw, s * CW + (h + 1) * hw)
                    us = slice(h * hw, (h + 1) * hw)
                    for k in range(KO):
                        sc_eng.tensor_tensor(
                            out=zt8[:, k, hs], in0=xts[:, k, hs],
                            in1=ub[:, us].bitcast(f32), op=ALU.mult)

            def emit_gc(g, ms=None):
                gs0 = g * CW
                for m in (range(MT) if ms is None else ms):
                    idx = g * MT + m
                    ps = psum.tile([P, CW], f32, name="ps", tag="ps", bufs=2)
                    lhsT = zt8[:, :, m * P:(m + 1) * P]
                    for b in range(CW // PB):
                        bs = slice(b * PB, (b + 1) * PB)
                        mvs = slice(gs0 + b * PB, gs0 + (b + 1) * PB)
                        if MM_MODE == "dr":
                            nc.tensor.matmul(
                                ps[:, bs], lhsT, zt8[:, :, mvs],
                                start=True, stop=True, perf_mode=DR)
                        else:
                            for k in range(KO):
                                nc.tensor.matmul(
                                    ps[:, bs], zt8[:, k, m * P:(m + 1) * P],
                                    zt8[:, k, mvs],
                                    start=(k == 0), stop=(k == KO - 1))
                    if g == NCH // 2 and DO_POS:
                        # pos diagonal: rows m*128+pp pair with columns
                        # 4096 + m*128 + pp  ->  diag of this subblock
                        pscr = scratch.tile([P, P], f32, name="pscr",
                                            tag="pscr", bufs=2)
                        nc.vector.scalar_tensor_tensor(
                            out=pscr[:], in0=ps[:, m * P:(m + 1) * P],
                            scalar=EXP_SCALE, in1=ident[:],
                            op0=ALU.mult, op1=ALU.mult,
                            accum_out=posd[:, m:m + 1])
                    col = acc[:, idx:idx + 1]
                    mode = DRAINS[idx] if DO_SCH else "A"
                    if mode == "A":
                        esc = escp.tile([P, CW], bf16, name="esc", tag="esc",
                                        bufs=3)
                        nc.scalar.activation(
                            esc[:], ps[:], AF.Exp, scale=EXP_SCALE,
                            accum_out=col)
                    else:
                        # pass1 frees the PSUM slot; pass2 reads the
                        # f32-dtyped bits tile with a plain AP
                        it = itp.tile([P, CW], f32, name="it", tag="it",
                                      bufs=2)
                        nc.vector.tensor_scalar(
                            out=it[:].bitcast(i32), in0=ps[:], scalar1=SCH_A,
                            scalar2=SCH_B, op0=ALU.mult, op1=ALU.add)
                        dm = itp.tile([P, CW], f32, name="dm", tag="dm",
                                      bufs=2)
                        nc.vector.tensor_scalar(
                            out=dm[:], in0=it[:], scalar1=1.0, scalar2=0.0,
                            op0=ALU.mult, op1=ALU.add, accum_out=col)

            # ---- pipeline ----
            # chunk0 prep, then gc0's FIRST tile immediately (so the ACT
            # exp chain — the wall — starts ~10us sooner; it would
            # otherwise queue behind chunk1's ones-matmuls on the tensor
            # engine), then chunk1 prep, rest of gc0, and gc_i
            # interleaved with chunk_{i+2} prep.
            emit_chunk(0)
            emit_uq(0)
            emit_gc(0, ms=[0])
            emit_chunk(1)
            emit_uq(1)
            emit_gc(0, ms=range(1, MT))
            for g in range(1, NCH):
                if g + 1 < NCH:
                    emit_chunk(g + 1)
                    emit_uq(g + 1)
                emit_gc(g)



            # ---- tail ----
            dsum = small.tile([P, MT], f32, name="dsum")
            nc.vector.tensor_reduce(
                dsum[:], acc[:].rearrange("p (m g) -> p m g", g=NCH),
                axis=mybir.AxisListType.X, op=ALU.add)
            dsub = small.tile([P, MT], f32, name="dsub")
            nc.vector.tensor_scalar_add(dsub[:], dsum[:], -E2)
            lnd = small.tile([P, MT], f32, name="lnd")
            nc.scalar.activation(lnd[:], dsub[:], AF.Ln)
            l1 = small.tile([P, 1], f32, name="l1")
            nc.vector.tensor_reduce(l1[:], lnd[:], axis=mybir.AxisListType.X,
                                    op=ALU.add)
            p1 = small.tile([P, 1], f32, name="p1")
            nc.vector.tensor_reduce(p1[:], posd[:], axis=mybir.AxisListType.X,
                                    op=ALU.add)
            comb = small.tile([P, 1], f32, name="comb")
            nc.vector.tensor_sub(comb[:], l1[:], p1[:])
            nc.sync.dma_start(out_d[:, :], comb[:])

    with tile.TileContext(nc) as tc:
        _emit(tc)
    nc.compile()
    return nc


def _get_nc():
    global _cached
    if _cached is None:
        _cached = _build()
    return _cached


def _make_in_maps(x_i, x_j):
    import ml_dtypes

    xall = np.concatenate(
        [np.asarray(x_i, dtype=np.float32), np.asarray(x_j, dtype=np.float32)],
        axis=0,
    )
    maps = []
    for c in range(NCORES):
        xc = np.roll(xall, -c * BLK, axis=0)
        xt = np.ascontiguousarray(xc.T).astype(ml_dtypes.bfloat16)
        maps.append({"xt": xt})
    return maps


def run(x_i, x_j, trace=False, tmpdir=None):
    from concourse import bass_utils

    nc = _get_nc()
    in_maps = _make_in_maps(x_i, x_j)
    res = bass_utils.run_bass_kernel_spmd(
        nc, in_maps, core_ids=list(range(NCORES)), trace=trace, tmpdir=tmpdir,
    )
    total = np.float32(0.0)
    for r in res.results:
        total += np.float32(np.sum(r["out"], dtype=np.float32))
    loss = np.float32(total / np.float32(R))
    return loss, res


def kernel(x_i, x_j):
    loss, _ = run(x_i, x_j, trace=False)
    return loss
